# revision 21
# baseline (speedup 1.0000x reference)
"""Trainium2 Bass kernel for nn_Attention_79645873537262.

Dense attention with per-head bias, key masking, sigmoid gate:
  t = x @ w_proj.T; per head: q,k,v
  a = softmax(scale*q@k.T + bias + mask); y = a@v
  y = sigmoid(x@w_g.T + b_g) * y;  out = y @ w_o.T + b_o

Sharding: tensor-parallel over heads, 2 heads per core on 8 cores.
Each core runs a fully independent program (no collectives); the host
sums the 8 partial o_proj outputs and adds b_o.

v3 design (all fp16 data path, PSUM f32; measured PE-bound ~94us):
- fp16 operands: LDWEIGHTS pipelines behind matmuls; MMs stream at
  ~215ns/512-col with back-to-back issue.
- Bias split per key-chunk kt: kt < KT_PE adds raw bias via PE identity
  matmul; kt >= KT_PE multiplies host-precomputed exp(bias) on DVE at
  fp16 2x rate. p = exp(s + b - 10*ln2) (shift cancels in softmax,
  keeps fp16 from overflowing; max s+b ~ 15).
- Scores for the 2 heads run concurrently (K=64 row tiles, pair
  measured at 386ns for both).
- AV matmuls lag their kt by 2 so the PE FIFO never head-blocks on the
  exp+mult chain.
- v transposed key-major by 32 transpose-DMAs on the sync queue (bias
  stream lives on gpsimd so the transpose train can't starve it);
  destinations are 16B-aligned 80-wide slots (unaligned transpose dsts
  corrupt neighboring columns).
- o_proj(qhalf 0) is interleaved into attention qhalf 1 (1 block per
  2 kt) to fill PE slack; the tail runs per-qq norm -> o_proj chains.
- Normalization: denom row (ones-column of the M=65 AV) -> DRAM
  round-trip broadcast -> reciprocal_approx_fast; gate fused as
  (tanh+1)*recip in one scalar_tensor_tensor (sigmoid(u) =
  0.5*(tanh(u/2)+1); the 0.5 is folded into w_o on host).
"""
import sys
import numpy as np

try:
    import concourse.bass as bass
except ImportError:
    sys.path.insert(0, "/opt/trn_rl_repo")
    import concourse.bass as bass

import concourse.tile as tile
from concourse import bacc, mybir
from concourse.bass_utils import run_bass_kernel_spmd

B, L, E, H = 1, 2048, 1024, 16
HW = E // H                # 64
SCALE = HW ** -0.5
N_CORES = 8
HPC = H // N_CORES         # 2 heads per core
C2 = HPC * HW              # 128
MASK_NEG = -60.0

f32 = mybir.dt.float32
f16 = mybir.dt.float16

NE = E // 128              # 8 contraction chunks
NKT = L // 128             # 16 key chunks of 128
KT_PE = 4                  # key chunks whose bias goes through the PE
AV_LAG = 2                 # kt lag between scores and AV matmuls
# log-domain shifts so p = exp(s + b - 10*ln2) never overflows f16
C_EXP = float(6 * np.log(2.0))   # applied inside the Exp activation
C_BM = float(4 * np.log(2.0))    # applied to the bias on host

_compiled = [None]


def _build():
    nc = bacc.Bacc("TRN2", target_bir_lowering=False, debug=False,
                   num_devices=N_CORES)

    xT_ap = nc.dram_tensor("xT", [E, L], f16, kind="ExternalInput").ap()
    wpT_ap = nc.dram_tensor("wpT", [E, 3 * C2], f16, kind="ExternalInput").ap()
    wgT_ap = nc.dram_tensor("wgT", [E, C2], f16, kind="ExternalInput").ap()
    bgt_ap = nc.dram_tensor("bgt", [C2, 1], f32, kind="ExternalInput").ap()
    woT_ap = nc.dram_tensor("woT", [C2, E], f16, kind="ExternalInput").ap()
    bm_ap = nc.dram_tensor("bm", [NKT, 128, HPC, L], f16,
                           kind="ExternalInput").ap()
    ident_ap = nc.dram_tensor("ident", [128, 128], f16, kind="ExternalInput").ap()
    ones_ap = nc.dram_tensor("onescols", [128, NKT * 2], f16,
                             kind="ExternalInput").ap()
    outT_ap = nc.dram_tensor("outT", [E, L], f16, kind="ExternalOutput").ap()

    AOP = mybir.AluOpType
    EXP = mybir.ActivationFunctionType.Exp

    with tile.TileContext(nc) as tc:
        from contextlib import ExitStack
        with ExitStack() as ctx:
            pers = ctx.enter_context(tc.tile_pool(name="pers", bufs=1))
            biasp = ctx.enter_context(tc.tile_pool(name="bias", bufs=4))
            pp = ctx.enter_context(tc.tile_pool(name="pp", bufs=10))
            pep = ctx.enter_context(tc.tile_pool(name="pep", bufs=4))
            nrm = ctx.enter_context(tc.tile_pool(name="nrm", bufs=2))
            dramp = ctx.enter_context(tc.tile_pool(name="dram", bufs=4, space="DRAM"))
            outp = ctx.enter_context(tc.tile_pool(name="outp", bufs=4))
            # PSUM: 8 banks = s(2 bufs x 2 banks) + y0/y1(1 buf x 2 banks each)
            sp = ctx.enter_context(tc.tile_pool(name="s", bufs=2, space="PSUM"))
            yp = ctx.enter_context(tc.tile_pool(name="y", bufs=1, space="PSUM"))

            # ---- input DMAs (sync queue): ones first (tiny RMW writes must
            # land before the v transpose copies), then proj-critical tensors
            v_all_early = pers.tile([128, NKT, 136], f16, tag="v_all")
            nc.gpsimd.memset(v_all_early[:, :, 64:65], 1.0)
            nc.gpsimd.memset(v_all_early[:, :, 132:133], 1.0)
            wpT_sb = [pers.tile([128, 3 * C2], f16, name=f"wpT{e}", tag=f"wpT{e}")
                      for e in range(NE)]
            xT_sb = [pers.tile([128, L], f16, name=f"xT{e}", tag=f"xT{e}")
                     for e in range(NE)]
            for e in range(NE):
                nc.sync.dma_start(wpT_sb[e], wpT_ap[e * 128:(e + 1) * 128, :])
                nc.sync.dma_start(xT_sb[e][:, 0:1024],
                                  xT_ap[e * 128:(e + 1) * 128, 0:1024])
            for e in range(NE):
                nc.sync.dma_start(xT_sb[e][:, 1024:2048],
                                  xT_ap[e * 128:(e + 1) * 128, 1024:2048])
            ident_sb = pers.tile([128, 128], f16, tag="ident")
            nc.sync.dma_start(ident_sb, ident_ap)
            wgT_sb = [pers.tile([128, C2], f16, name=f"wgT{e}", tag=f"wgT{e}")
                      for e in range(NE)]
            for e in range(NE):
                nc.sync.dma_start(wgT_sb[e], wgT_ap[e * 128:(e + 1) * 128, :])
            bgt_sb = pers.tile([C2, 1], f32, tag="bgt")
            nc.sync.dma_start(bgt_sb, bgt_ap)
            woT_sb = pers.tile([C2, E], f16, tag="woT")
            nc.sync.dma_start(woT_sb, woT_ap)
            # v layout [128 keys, kt, 136]: h0 = [v 0:64 | ones 64], h1 =
            # [v 68:132 | ones 132]; pads keep the ones columns on their own
            # 8-byte lines (the sub-512B ones-DMA does read-modify-write and
            # races DVE copies that share a line). Ones DMAs issued first.
            v_all = v_all_early

            # Act spline-table warmup (exp_and_others: Exp + Tanh)
            warm = pers.tile([C2, 1], f32, tag="warm")
            nc.scalar.activation(warm, bgt_sb, EXP)
            cexp_sb = pers.tile([128, 1], f32, tag="cexp")
            nc.gpsimd.memset(cexp_sb, -C_EXP)

            q01 = pers.tile([128, L], f16, tag="q01")
            k01 = pers.tile([128, L], f16, tag="k01")
            vT01 = pers.tile([128, L], f16, tag="vT01")
            g01 = pers.tile([128, L], f16, tag="g01")
            ygT = pers.tile([128, L], f16, tag="ygT")

            # ---------------- proj ----------------
            dests = {0: q01, 1: k01, 2: vT01}

            def proj_lh(f, lh, drain_eng=None):
                ps = sp.tile([128, 2, 512], f32, name=f"pj{f}_{lh}", tag="s")
                for e in range(NE):
                    w = wpT_sb[e][:, f * 128:(f + 1) * 128]
                    for ltq in range(2):
                        nc.tensor.matmul(
                            ps[:, ltq, :], w,
                            xT_sb[e][:, lh * 1024 + ltq * 512:
                                     lh * 1024 + (ltq + 1) * 512],
                            start=(e == 0), stop=(e == NE - 1))
                nc.vector.tensor_copy(
                    dests[f][:, lh * 1024:(lh + 1) * 1024], ps)

            def gate_lh(lh):
                ps = sp.tile([128, 2, 512], f32, name=f"pg{lh}", tag="s")
                for e in range(NE):
                    for ltq in range(2):
                        nc.tensor.matmul(
                            ps[:, ltq, :], wgT_sb[e],
                            xT_sb[e][:, lh * 1024 + ltq * 512:
                                     lh * 1024 + (ltq + 1) * 512],
                            start=(e == 0), stop=(e == NE - 1))
                nc.scalar.activation(
                    g01[:, lh * 1024:(lh + 1) * 1024], ps,
                    mybir.ActivationFunctionType.Tanh,
                    bias=bgt_sb, scale=0.5)

            # v (both halves) -> transposes; k (both); q lh0 only (qh0's
            # scores need q cols 0:1024; q-lh1 + gate interleave into qh0)
            def vtrans(kt):
                kts = slice(kt * 128, (kt + 1) * 128)
                tr = sp.tile([128, 128], f16, name=f"tr{kt}", tag="s")
                nc.tensor.transpose(tr, vT01[:, kts], ident_sb)
                nc.vector.tensor_copy(v_all[:, kt, 0:64], tr[:, 0:64])
                nc.vector.tensor_copy(v_all[:, kt, 68:132], tr[:, 64:128])

            proj_lh(2, 0)
            proj_lh(2, 1)
            proj_lh(1, 0)
            proj_lh(1, 1)
            proj_lh(0, 0)


            # ---------------- attention ----------------
            def emit_avs(y_t, pl, k0):
                for qq in range(2):
                    for h in range(2):
                        nc.tensor.matmul(
                            y_t[qq][:, h, :],
                            v_all[:, k0, h * 68:h * 68 + 65],
                            pl[qq][:, h, :],
                            start=(k0 == 0), stop=(k0 == NKT - 1))

            def attention_pass(qhalf, extras=None, pre=None):
                y_t = [yp.tile([65, 2, 512], f32, name=f"y{qhalf}_{qq}",
                               tag=f"y{qq}") for qq in range(2)]
                pend = []
                for kt in range(NKT):
                    if pre is not None and kt in pre:
                        pre[kt]()
                    kts = slice(kt * 128, (kt + 1) * 128)
                    bt = biasp.tile([128, 2, 1024], f16,
                                    name=f"bt{qhalf}_{kt}", tag="bias")
                    nc.gpsimd.dma_start(
                        bt, bm_ap[kt, :, :, qhalf * 1024:(qhalf + 1) * 1024])
                    p_kt = []
                    for qq in range(2):
                        qs = slice(qhalf * 1024 + qq * 512,
                                   qhalf * 1024 + (qq + 1) * 512)
                        s_t = sp.tile([128, 2, 512], f32,
                                      name=f"s{qhalf}_{kt}_{qq}", tag="s")
                        pe_bias = kt < KT_PE
                        if pe_bias:
                            for h in range(2):
                                nc.tensor.matmul(
                                    s_t[:, h, :], ident_sb,
                                    bt[:, h, qq * 512:(qq + 1) * 512],
                                    start=True, stop=False)
                        for h in range(2):
                            hb = h * 64
                            nc.tensor.matmul(
                                s_t[:, h, :],
                                k01[hb:hb + 64, kts],
                                q01[hb:hb + 64, qs],
                                start=not pe_bias, stop=True)
                        p_t = pp.tile([128, 2, 512], f16,
                                      name=f"p{qhalf}_{kt}_{qq}", tag="p")
                        if pe_bias:
                            nc.scalar.activation(p_t, s_t, EXP)
                        else:
                            pe_t = pep.tile([128, 2, 512], f16,
                                            name=f"pe{qhalf}_{kt}_{qq}", tag="pe")
                            nc.scalar.activation(pe_t, s_t, EXP, bias=cexp_sb)
                            nc.vector.tensor_mul(
                                p_t, pe_t, bt[:, :, qq * 512:(qq + 1) * 512])
                        p_kt.append(p_t)
                    if extras is not None and kt in extras:
                        extras[kt]()
                    pend.append((p_kt, kt))
                    if len(pend) > AV_LAG:
                        pl, k0 = pend.pop(0)
                        emit_avs(y_t, pl, k0)
                for pl, k0 in pend:
                    emit_avs(y_t, pl, k0)
                return y_t

            def norm_pass(qhalf, qq, y_t):
                qt = qhalf * 2 + qq
                qsl = slice(qt * 512, (qt + 1) * 512)
                sums = nrm.tile([65, 2, 512], f32, name=f"sm{qhalf}_{qq}",
                                tag="sums")
                nc.vector.tensor_copy(sums[64:65, :, :], y_t[64:65, :, :])
                dscr = dramp.tile([1, 2, 512], f32, name=f"dscr{qhalf}_{qq}",
                                  tag="dscr")
                nc.gpsimd.dma_start(dscr, sums[64:65, :, :])
                rbs = nrm.tile([128, 512], f32, name=f"rbs{qhalf}_{qq}", tag="rbs")
                nc.gpsimd.dma_start(
                    rbs[0:64, :], dscr[0:1, 0, :].partition_broadcast(64))
                nc.gpsimd.dma_start(
                    rbs[64:128, :], dscr[0:1, 1, :].partition_broadcast(64))
                rb = nrm.tile([128, 512], f32, name=f"rb{qhalf}_{qq}", tag="rb")
                nc.vector.reciprocal_approx_fast(rb, rbs)
                # grb = (tanh + 1) * (1/denom); the 0.5 lives in woT
                grb = nrm.tile([128, 512], f16, name=f"grb{qhalf}_{qq}", tag="grb")
                nc.vector.scalar_tensor_tensor(
                    grb, g01[:, qsl], 1.0, rb, AOP.add, AOP.mult)
                nc.vector.tensor_mul(ygT[0:64, qsl], y_t[0:64, 0, :], grb[0:64, :])
                yg1 = nrm.tile([64, 512], f16, name=f"yg1{qhalf}_{qq}", tag="yg1")
                nc.vector.tensor_mul(yg1, y_t[0:64, 1, :], grb[64:128, :])
                nc.gpsimd.dma_start(ygT[64:128, qsl], yg1)

            def oproj_block(qhalf, eo):
                # both qq halves of one eo slice: 2 MMs + drain + out-DMA
                ps = sp.tile([128, 2, 512], f32, name=f"po{qhalf}_{eo}", tag="s")
                for qq in range(2):
                    qt = qhalf * 2 + qq
                    nc.tensor.matmul(
                        ps[:, qq, :],
                        woT_sb[:, eo * 128:(eo + 1) * 128],
                        ygT[:, qt * 512:(qt + 1) * 512],
                        start=True, stop=True)
                ot = outp.tile([128, 2, 512], f16, name=f"ot{qhalf}_{eo}",
                               tag="ot")
                nc.vector.tensor_copy(ot, ps)
                nc.sync.dma_start(
                    outT_ap[eo * 128:(eo + 1) * 128,
                            qhalf * 1024:(qhalf + 1) * 1024], ot)

            def oproj_qq(qhalf, qq, eo):
                qt = qhalf * 2 + qq
                ps = sp.tile([128, 512], f32, name=f"pq{qhalf}_{qq}_{eo}",
                             tag="s")
                nc.tensor.matmul(
                    ps, woT_sb[:, eo * 128:(eo + 1) * 128],
                    ygT[:, qt * 512:(qt + 1) * 512], start=True, stop=True)
                ot = outp.tile([128, 512], f16, name=f"oq{qhalf}_{qq}_{eo}",
                               tag="ot")
                if eo % 2 == 0:
                    nc.vector.tensor_copy(ot, ps)
                else:
                    nc.scalar.copy(ot, ps)
                nc.sync.dma_start(
                    outT_ap[eo * 128:(eo + 1) * 128,
                            qt * 512:(qt + 1) * 512], ot)

            extras0 = {2: lambda: proj_lh(0, 1),
                       6: lambda: gate_lh(0),
                       10: lambda: gate_lh(1)}
            pre0 = {kt: (lambda kt=kt: vtrans(kt)) for kt in range(NKT)}
            y_q0 = attention_pass(0, extras=extras0, pre=pre0)
            norm_pass(0, 0, y_q0[0])
            norm_pass(0, 1, y_q0[1])
            # o_proj(qh0) interleaved into attention(qh1): 1 block per kt
            extras = {5 + j: (lambda j=j: oproj_block(0, j))
                      for j in range(NE)}
            y_q1 = attention_pass(1, extras=extras)
            norm_pass(1, 0, y_q1[0])
            norm_pass(1, 1, y_q1[1])
            for eo in range(NE):
                oproj_qq(1, 0, eo)
                oproj_qq(1, 1, eo)

    nc.compile()
    return nc


def kernel(x, mask, bias, w_proj, w_o, b_o, w_g, b_g):
    x = np.asarray(x, dtype=np.float32)
    mask = np.asarray(mask)
    bias = np.asarray(bias, dtype=np.float32)
    w_proj = np.asarray(w_proj, dtype=np.float32)
    w_o = np.asarray(w_o, dtype=np.float32)
    b_o = np.asarray(b_o, dtype=np.float32)
    w_g = np.asarray(w_g, dtype=np.float32)
    b_g = np.asarray(b_g, dtype=np.float32)

    if _compiled[0] is None:
        _compiled[0] = _build()
    nc = _compiled[0]

    xT = np.ascontiguousarray(x[0].T).astype(np.float16)      # [E, L]
    mask_add = np.where(mask[0], 0.0, MASK_NEG).astype(np.float32)  # [L]
    ident = np.eye(128, dtype=np.float16)
    onescols = np.ones((128, NKT * 2), dtype=np.float16)

    in_maps = []
    for c in range(N_CORES):
        heads = [c * HPC + i for i in range(HPC)]
        wpT = np.empty((E, 3 * C2), dtype=np.float16)
        for i, h in enumerate(heads):
            r0 = h * 3 * HW
            wpT[:, 0 * C2 + i * HW: 0 * C2 + (i + 1) * HW] = \
                (w_proj[r0: r0 + HW].T * SCALE).astype(np.float16)   # q
            wpT[:, 1 * C2 + i * HW: 1 * C2 + (i + 1) * HW] = \
                w_proj[r0 + HW: r0 + 2 * HW].T.astype(np.float16)    # k
            wpT[:, 2 * C2 + i * HW: 2 * C2 + (i + 1) * HW] = \
                w_proj[r0 + 2 * HW: r0 + 3 * HW].T.astype(np.float16)  # v
        # biasmix [NKT, 128, HPC, L]: raw bias (kt < KT_PE) else exp(bias)
        biasT = np.ascontiguousarray(
            bias[0, :, :, heads].transpose(0, 2, 1))          # [HPC, Lk, Lq]
        biasT += mask_add[None, :, None]
        bm = biasT.reshape(HPC, NKT, 128, L).transpose(1, 2, 0, 3)  # kt,k,h,q
        bmix = np.empty((NKT, 128, HPC, L), dtype=np.float16)
        bmix[:KT_PE] = bm[:KT_PE] - (C_EXP + C_BM)
        bmix[KT_PE:] = np.exp(bm[KT_PE:] - C_BM)
        cols = slice(c * C2, (c + 1) * C2)
        wgT = np.ascontiguousarray(w_g[cols, :].T).astype(np.float16)  # [E, C2]
        bgt = np.ascontiguousarray(0.5 * b_g[cols, None]).astype(np.float32)
        woT = np.ascontiguousarray(0.5 * w_o[:, cols].T).astype(np.float16)
        in_maps.append({
            "xT": xT, "wpT": wpT, "bm": bmix, "wgT": wgT,
            "bgt": bgt, "woT": woT, "ident": ident, "onescols": onescols,
        })

    res = run_bass_kernel_spmd(nc, in_maps, list(range(N_CORES)))
    acc = res.results[0]["outT"].astype(np.float64)
    for c in range(1, N_CORES):
        acc += res.results[c]["outT"]
    out = acc.T.astype(np.float32) + b_o[None, :]
    return out[None]  # [B, L, E]


# revision 23
# speedup vs baseline: 1.0780x; 1.0780x over previous
"""Trainium2 Bass kernel for nn_Attention_79645873537262.

Dense attention with per-head bias, key masking, sigmoid gate:
  t = x @ w_proj.T; per head: q,k,v
  a = softmax(scale*q@k.T + bias + mask); y = a@v
  y = sigmoid(x@w_g.T + b_g) * y;  out = y @ w_o.T + b_o

Sharding: tensor-parallel over heads, 2 heads per core on 8 cores.
Each core runs a fully independent program (no collectives); the host
sums the 8 partial o_proj outputs and adds b_o.

v3 design (all fp16 data path, PSUM f32; measured PE-bound ~94us):
- fp16 operands: LDWEIGHTS pipelines behind matmuls; MMs stream at
  ~215ns/512-col with back-to-back issue.
- Bias split per key-chunk kt: kt < KT_PE adds raw bias via PE identity
  matmul; kt >= KT_PE multiplies host-precomputed exp(bias) on DVE at
  fp16 2x rate. p = exp(s + b - 10*ln2) (shift cancels in softmax,
  keeps fp16 from overflowing; max s+b ~ 15).
- Scores for the 2 heads run concurrently (K=64 row tiles, pair
  measured at 386ns for both).
- AV matmuls lag their kt by 2 so the PE FIFO never head-blocks on the
  exp+mult chain.
- v transposed key-major by 32 transpose-DMAs on the sync queue (bias
  stream lives on gpsimd so the transpose train can't starve it);
  destinations are 16B-aligned 80-wide slots (unaligned transpose dsts
  corrupt neighboring columns).
- o_proj(qhalf 0) is interleaved into attention qhalf 1 (1 block per
  2 kt) to fill PE slack; the tail runs per-qq norm -> o_proj chains.
- Normalization: denom row (ones-column of the M=65 AV) -> DRAM
  round-trip broadcast -> reciprocal_approx_fast; gate fused as
  (tanh+1)*recip in one scalar_tensor_tensor (sigmoid(u) =
  0.5*(tanh(u/2)+1); the 0.5 is folded into w_o on host).
"""
import sys
import numpy as np

try:
    import concourse.bass as bass
except ImportError:
    sys.path.insert(0, "/opt/trn_rl_repo")
    import concourse.bass as bass

import concourse.tile as tile
from concourse import bacc, mybir
from concourse.bass_utils import run_bass_kernel_spmd

B, L, E, H = 1, 2048, 1024, 16
HW = E // H                # 64
SCALE = HW ** -0.5
N_CORES = 8
HPC = H // N_CORES         # 2 heads per core
C2 = HPC * HW              # 128
MASK_NEG = -60.0

f32 = mybir.dt.float32
f16 = mybir.dt.float16

NE = E // 128              # 8 contraction chunks
NKT = L // 128             # 16 key chunks of 128
KT_PE = 4                  # key chunks whose bias goes through the PE
AV_LAG = 2                 # kt lag between scores and AV matmuls
# log-domain shifts so p = exp(s + b - 10*ln2) never overflows f16
C_EXP = float(6 * np.log(2.0))   # applied inside the Exp activation
C_BM = float(4 * np.log(2.0))    # applied to the bias on host

_compiled = [None]


def _build():
    nc = bacc.Bacc("TRN2", target_bir_lowering=False, debug=False,
                   num_devices=N_CORES)

    xT_ap = nc.dram_tensor("xT", [E, L], f16, kind="ExternalInput").ap()
    wpT_ap = nc.dram_tensor("wpT", [E, 3 * C2], f16, kind="ExternalInput").ap()
    wgT_ap = nc.dram_tensor("wgT", [E, C2], f16, kind="ExternalInput").ap()
    bgt_ap = nc.dram_tensor("bgt", [C2, 1], f32, kind="ExternalInput").ap()
    woT_ap = nc.dram_tensor("woT", [C2, E], f16, kind="ExternalInput").ap()
    bm_ap = nc.dram_tensor("bm", [NKT, 128, HPC, L], f16,
                           kind="ExternalInput").ap()
    ident_ap = nc.dram_tensor("ident", [128, 128], f16, kind="ExternalInput").ap()
    ones_ap = nc.dram_tensor("onescols", [128, NKT * 2], f16,
                             kind="ExternalInput").ap()
    outT_ap = nc.dram_tensor("outT", [E, L], f16, kind="ExternalOutput").ap()

    AOP = mybir.AluOpType
    EXP = mybir.ActivationFunctionType.Exp

    with tile.TileContext(nc) as tc:
        from contextlib import ExitStack
        with ExitStack() as ctx:
            pers = ctx.enter_context(tc.tile_pool(name="pers", bufs=1))
            biasp = ctx.enter_context(tc.tile_pool(name="bias", bufs=4))
            pp = ctx.enter_context(tc.tile_pool(name="pp", bufs=10))
            pep = ctx.enter_context(tc.tile_pool(name="pep", bufs=4))
            nrm = ctx.enter_context(tc.tile_pool(name="nrm", bufs=2))
            dramp = ctx.enter_context(tc.tile_pool(name="dram", bufs=4, space="DRAM"))
            outp = ctx.enter_context(tc.tile_pool(name="outp", bufs=4))
            # PSUM: 8 banks = s(2 bufs x 2 banks) + y0/y1(1 buf x 2 banks each)
            sp = ctx.enter_context(tc.tile_pool(name="s", bufs=2, space="PSUM"))
            yp = ctx.enter_context(tc.tile_pool(name="y", bufs=1, space="PSUM"))

            # ---- input DMAs (sync queue): ones first (tiny RMW writes must
            # land before the v transpose copies), then proj-critical tensors
            v_all_early = pers.tile([128, NKT, 2, 80], f16, tag="v_all")
            nc.gpsimd.memset(v_all_early[:, :, :, 64:65], 1.0)
            wpT_sb = [pers.tile([128, 3 * C2], f16, name=f"wpT{e}", tag=f"wpT{e}")
                      for e in range(NE)]
            xT_sb = [pers.tile([128, L], f16, name=f"xT{e}", tag=f"xT{e}")
                     for e in range(NE)]
            for e in range(NE):
                nc.sync.dma_start(wpT_sb[e], wpT_ap[e * 128:(e + 1) * 128, :])
                nc.sync.dma_start(xT_sb[e][:, 0:1024],
                                  xT_ap[e * 128:(e + 1) * 128, 0:1024])
            for e in range(NE):
                nc.sync.dma_start(xT_sb[e][:, 1024:2048],
                                  xT_ap[e * 128:(e + 1) * 128, 1024:2048])
            ident_sb = pers.tile([128, 128], f16, tag="ident")
            nc.sync.dma_start(ident_sb, ident_ap)
            wgT_sb = [pers.tile([128, C2], f16, name=f"wgT{e}", tag=f"wgT{e}")
                      for e in range(NE)]
            for e in range(NE):
                nc.sync.dma_start(wgT_sb[e], wgT_ap[e * 128:(e + 1) * 128, :])
            bgt_sb = pers.tile([C2, 1], f32, tag="bgt")
            nc.sync.dma_start(bgt_sb, bgt_ap)
            woT_sb = pers.tile([C2, E], f16, tag="woT")
            nc.sync.dma_start(woT_sb, woT_ap)
            # v layout [128 keys, kt, 136]: h0 = [v 0:64 | ones 64], h1 =
            # [v 68:132 | ones 132]; pads keep the ones columns on their own
            # 8-byte lines (the sub-512B ones-DMA does read-modify-write and
            # races DVE copies that share a line). Ones DMAs issued first.
            v_all = v_all_early

            # Act spline-table warmup (exp_and_others: Exp + Tanh)
            warm = pers.tile([C2, 1], f32, tag="warm")
            nc.scalar.activation(warm, bgt_sb, EXP)
            cexp_sb = pers.tile([128, 1], f32, tag="cexp")
            nc.gpsimd.memset(cexp_sb, -C_EXP)

            q01 = pers.tile([128, L], f16, tag="q01")
            k01 = pers.tile([128, L], f16, tag="k01")
            vT01 = pers.tile([128, L], f16, tag="vT01")
            g01 = pers.tile([128, L], f16, tag="g01")
            ygT = pers.tile([128, L], f16, tag="ygT")

            # ---------------- proj ----------------
            dests = {0: q01, 1: k01, 2: vT01}

            def proj_lh(f, lh, drain_eng=None):
                ps = sp.tile([128, 2, 512], f32, name=f"pj{f}_{lh}", tag="s")
                for e in range(NE):
                    w = wpT_sb[e][:, f * 128:(f + 1) * 128]
                    for ltq in range(2):
                        nc.tensor.matmul(
                            ps[:, ltq, :], w,
                            xT_sb[e][:, lh * 1024 + ltq * 512:
                                     lh * 1024 + (ltq + 1) * 512],
                            start=(e == 0), stop=(e == NE - 1))
                nc.vector.tensor_copy(
                    dests[f][:, lh * 1024:(lh + 1) * 1024], ps)

            def gate_lh(lh):
                ps = sp.tile([128, 2, 512], f32, name=f"pg{lh}", tag="s")
                for e in range(NE):
                    for ltq in range(2):
                        nc.tensor.matmul(
                            ps[:, ltq, :], wgT_sb[e],
                            xT_sb[e][:, lh * 1024 + ltq * 512:
                                     lh * 1024 + (ltq + 1) * 512],
                            start=(e == 0), stop=(e == NE - 1))
                nc.scalar.activation(
                    g01[:, lh * 1024:(lh + 1) * 1024], ps,
                    mybir.ActivationFunctionType.Tanh,
                    bias=bgt_sb, scale=0.5)

            # v (both halves) -> transposes; k (both); q lh0 only (qh0's
            # scores need q cols 0:1024; q-lh1 + gate interleave into qh0)
            proj_lh(2, 0)
            proj_lh(2, 1)
            for kt in range(NKT):
                kts = slice(kt * 128, (kt + 1) * 128)
                nc.sync.dma_start_transpose(v_all[:, kt, 0, 0:64],
                                            vT01[0:64, kts])
                nc.sync.dma_start_transpose(v_all[:, kt, 1, 0:64],
                                            vT01[64:128, kts])
            proj_lh(1, 0)
            proj_lh(1, 1)
            proj_lh(0, 0)


            # ---------------- attention ----------------
            def emit_avs(y_t, pl, k0):
                for qq in range(2):
                    for h in range(2):
                        nc.tensor.matmul(
                            y_t[qq][:, h, :],
                            v_all[:, k0, h, 0:65],
                            pl[qq][:, h, :],
                            start=(k0 == 0), stop=(k0 == NKT - 1))

            def attention_pass(qhalf, extras=None, pre=None):
                y_t = [yp.tile([65, 2, 512], f32, name=f"y{qhalf}_{qq}",
                               tag=f"y{qq}") for qq in range(2)]
                pend = []
                for kt in range(NKT):
                    if pre is not None and kt in pre:
                        pre[kt]()
                    kts = slice(kt * 128, (kt + 1) * 128)
                    bt = biasp.tile([128, 2, 1024], f16,
                                    name=f"bt{qhalf}_{kt}", tag="bias")
                    nc.gpsimd.dma_start(
                        bt, bm_ap[kt, :, :, qhalf * 1024:(qhalf + 1) * 1024])
                    p_kt = []
                    for qq in range(2):
                        qs = slice(qhalf * 1024 + qq * 512,
                                   qhalf * 1024 + (qq + 1) * 512)
                        s_t = sp.tile([128, 2, 512], f32,
                                      name=f"s{qhalf}_{kt}_{qq}", tag="s")
                        pe_bias = kt < KT_PE
                        if pe_bias:
                            for h in range(2):
                                nc.tensor.matmul(
                                    s_t[:, h, :], ident_sb,
                                    bt[:, h, qq * 512:(qq + 1) * 512],
                                    start=True, stop=False)
                        for h in range(2):
                            hb = h * 64
                            nc.tensor.matmul(
                                s_t[:, h, :],
                                k01[hb:hb + 64, kts],
                                q01[hb:hb + 64, qs],
                                start=not pe_bias, stop=True)
                        p_t = pp.tile([128, 2, 512], f16,
                                      name=f"p{qhalf}_{kt}_{qq}", tag="p")
                        if pe_bias:
                            nc.scalar.activation(p_t, s_t, EXP)
                        else:
                            pe_t = pep.tile([128, 2, 512], f16,
                                            name=f"pe{qhalf}_{kt}_{qq}", tag="pe")
                            nc.scalar.activation(pe_t, s_t, EXP, bias=cexp_sb)
                            nc.vector.tensor_mul(
                                p_t, pe_t, bt[:, :, qq * 512:(qq + 1) * 512])
                        p_kt.append(p_t)
                    if extras is not None and kt in extras:
                        extras[kt]()
                    pend.append((p_kt, kt))
                    if len(pend) > AV_LAG:
                        pl, k0 = pend.pop(0)
                        emit_avs(y_t, pl, k0)
                for pl, k0 in pend:
                    emit_avs(y_t, pl, k0)
                return y_t

            def norm_a(qhalf, qq, y_t):
                qt = qhalf * 2 + qq
                qsl = slice(qt * 512, (qt + 1) * 512)
                sums = nrm.tile([65, 2, 512], f32, name=f"sm{qhalf}_{qq}",
                                tag="sums")
                nc.vector.tensor_copy(sums[64:65, :, :], y_t[64:65, :, :])
                dscr = dramp.tile([1, 2, 512], f32, name=f"dscr{qhalf}_{qq}",
                                  tag="dscr")
                nc.gpsimd.dma_start(dscr, sums[64:65, :, :])
                rbs = nrm.tile([128, 512], f32, name=f"rbs{qhalf}_{qq}", tag="rbs")
                nc.gpsimd.dma_start(
                    rbs[0:64, :], dscr[0:1, 0, :].partition_broadcast(64))
                nc.gpsimd.dma_start(
                    rbs[64:128, :], dscr[0:1, 1, :].partition_broadcast(64))
                rb = nrm.tile([128, 512], f32, name=f"rb{qhalf}_{qq}", tag="rb")
                nc.vector.reciprocal_approx_fast(rb, rbs)
                # grb = (tanh + 1) * (1/denom); the 0.5 lives in woT
                grb = nrm.tile([128, 512], f16, name=f"grb{qhalf}_{qq}", tag="grb")
                nc.vector.scalar_tensor_tensor(
                    grb, g01[:, qsl], 1.0, rb, AOP.add, AOP.mult)
                return grb

            def norm_b(qhalf, qq, y_t, grb):
                qt = qhalf * 2 + qq
                qsl = slice(qt * 512, (qt + 1) * 512)
                nc.vector.tensor_mul(ygT[0:64, qsl], y_t[0:64, 0, :], grb[0:64, :])
                yg1 = nrm.tile([64, 512], f16, name=f"yg1{qhalf}_{qq}", tag="yg1")
                nc.vector.tensor_mul(yg1, y_t[0:64, 1, :], grb[64:128, :])
                nc.sync.dma_start(ygT[64:128, qsl], yg1)

            def oproj_block(qhalf, eo):
                # both qq halves of one eo slice: 2 MMs + drain + out-DMA
                ps = sp.tile([128, 2, 512], f32, name=f"po{qhalf}_{eo}", tag="s")
                for qq in range(2):
                    qt = qhalf * 2 + qq
                    nc.tensor.matmul(
                        ps[:, qq, :],
                        woT_sb[:, eo * 128:(eo + 1) * 128],
                        ygT[:, qt * 512:(qt + 1) * 512],
                        start=True, stop=True)
                ot = outp.tile([128, 2, 512], f16, name=f"ot{qhalf}_{eo}",
                               tag="ot")
                nc.vector.tensor_copy(ot, ps)
                nc.sync.dma_start(
                    outT_ap[eo * 128:(eo + 1) * 128,
                            qhalf * 1024:(qhalf + 1) * 1024], ot)

            def oproj_qq(qhalf, qq, eo):
                qt = qhalf * 2 + qq
                ps = sp.tile([128, 512], f32, name=f"pq{qhalf}_{qq}_{eo}",
                             tag="s")
                nc.tensor.matmul(
                    ps, woT_sb[:, eo * 128:(eo + 1) * 128],
                    ygT[:, qt * 512:(qt + 1) * 512], start=True, stop=True)
                ot = outp.tile([128, 512], f16, name=f"oq{qhalf}_{qq}_{eo}",
                               tag="ot")
                if eo % 2 == 0:
                    nc.vector.tensor_copy(ot, ps)
                else:
                    nc.scalar.copy(ot, ps)
                nc.sync.dma_start(
                    outT_ap[eo * 128:(eo + 1) * 128,
                            qt * 512:(qt + 1) * 512], ot)

            extras0 = {2: lambda: proj_lh(0, 1),
                       8: lambda: gate_lh(0)}
            y_q0 = attention_pass(0, extras=extras0)
            g00 = norm_a(0, 0, y_q0[0])
            g01_ = norm_a(0, 1, y_q0[1])
            norm_b(0, 0, y_q0[0], g00)
            norm_b(0, 1, y_q0[1], g01_)
            # o_proj(qh0) + gate lh1 interleaved into attention(qh1)
            extras = {5 + j: (lambda j=j: oproj_block(0, j))
                      for j in range(NE)}
            extras[2] = lambda: gate_lh(1)
            y_q1 = attention_pass(1, extras=extras)
            g10 = norm_a(1, 0, y_q1[0])
            g11 = norm_a(1, 1, y_q1[1])
            norm_b(1, 0, y_q1[0], g10)
            norm_b(1, 1, y_q1[1], g11)
            for eo in range(NE):
                oproj_qq(1, 0, eo)
                oproj_qq(1, 1, eo)

    nc.compile()
    return nc


def kernel(x, mask, bias, w_proj, w_o, b_o, w_g, b_g):
    x = np.asarray(x, dtype=np.float32)
    mask = np.asarray(mask)
    bias = np.asarray(bias, dtype=np.float32)
    w_proj = np.asarray(w_proj, dtype=np.float32)
    w_o = np.asarray(w_o, dtype=np.float32)
    b_o = np.asarray(b_o, dtype=np.float32)
    w_g = np.asarray(w_g, dtype=np.float32)
    b_g = np.asarray(b_g, dtype=np.float32)

    if _compiled[0] is None:
        _compiled[0] = _build()
    nc = _compiled[0]

    xT = np.ascontiguousarray(x[0].T).astype(np.float16)      # [E, L]
    mask_add = np.where(mask[0], 0.0, MASK_NEG).astype(np.float32)  # [L]
    ident = np.eye(128, dtype=np.float16)
    onescols = np.ones((128, NKT * 2), dtype=np.float16)

    in_maps = []
    for c in range(N_CORES):
        heads = [c * HPC + i for i in range(HPC)]
        wpT = np.empty((E, 3 * C2), dtype=np.float16)
        for i, h in enumerate(heads):
            r0 = h * 3 * HW
            wpT[:, 0 * C2 + i * HW: 0 * C2 + (i + 1) * HW] = \
                (w_proj[r0: r0 + HW].T * SCALE).astype(np.float16)   # q
            wpT[:, 1 * C2 + i * HW: 1 * C2 + (i + 1) * HW] = \
                w_proj[r0 + HW: r0 + 2 * HW].T.astype(np.float16)    # k
            wpT[:, 2 * C2 + i * HW: 2 * C2 + (i + 1) * HW] = \
                w_proj[r0 + 2 * HW: r0 + 3 * HW].T.astype(np.float16)  # v
        # biasmix [NKT, 128, HPC, L]: raw bias (kt < KT_PE) else exp(bias)
        biasT = np.ascontiguousarray(
            bias[0, :, :, heads].transpose(0, 2, 1))          # [HPC, Lk, Lq]
        biasT += mask_add[None, :, None]
        bm = biasT.reshape(HPC, NKT, 128, L).transpose(1, 2, 0, 3)  # kt,k,h,q
        bmix = np.empty((NKT, 128, HPC, L), dtype=np.float16)
        bmix[:KT_PE] = bm[:KT_PE] - (C_EXP + C_BM)
        bmix[KT_PE:] = np.exp(bm[KT_PE:] - C_BM)
        cols = slice(c * C2, (c + 1) * C2)
        wgT = np.ascontiguousarray(w_g[cols, :].T).astype(np.float16)  # [E, C2]
        bgt = np.ascontiguousarray(0.5 * b_g[cols, None]).astype(np.float32)
        woT = np.ascontiguousarray(0.5 * w_o[:, cols].T).astype(np.float16)
        in_maps.append({
            "xT": xT, "wpT": wpT, "bm": bmix, "wgT": wgT,
            "bgt": bgt, "woT": woT, "ident": ident, "onescols": onescols,
        })

    res = run_bass_kernel_spmd(nc, in_maps, list(range(N_CORES)))
    acc = res.results[0]["outT"].astype(np.float64)
    for c in range(1, N_CORES):
        acc += res.results[c]["outT"]
    out = acc.T.astype(np.float32) + b_o[None, :]
    return out[None]  # [B, L, E]


# revision 24
# speedup vs baseline: 1.1896x; 1.1035x over previous
"""Trainium2 Bass kernel for nn_Attention_79645873537262.

Dense attention with per-head bias, key masking, sigmoid gate:
  t = x @ w_proj.T; per head: q,k,v
  a = softmax(scale*q@k.T + bias + mask); y = a@v
  y = sigmoid(x@w_g.T + b_g) * y;  out = y @ w_o.T + b_o

Sharding: tensor-parallel over heads, 2 heads per core on 8 cores.
Each core runs a fully independent program (no collectives); the host
sums the 8 partial o_proj outputs and adds b_o.

v3 design (all fp16 data path, PSUM f32; measured PE-bound ~94us):
- fp16 operands: LDWEIGHTS pipelines behind matmuls; MMs stream at
  ~215ns/512-col with back-to-back issue.
- Bias split per key-chunk kt: kt < KT_PE adds raw bias via PE identity
  matmul; kt >= KT_PE multiplies host-precomputed exp(bias) on DVE at
  fp16 2x rate. p = exp(s + b - 10*ln2) (shift cancels in softmax,
  keeps fp16 from overflowing; max s+b ~ 15).
- Scores for the 2 heads run concurrently (K=64 row tiles, pair
  measured at 386ns for both).
- AV matmuls lag their kt by 2 so the PE FIFO never head-blocks on the
  exp+mult chain.
- v transposed key-major by 32 transpose-DMAs on the sync queue (bias
  stream lives on gpsimd so the transpose train can't starve it);
  destinations are 16B-aligned 80-wide slots (unaligned transpose dsts
  corrupt neighboring columns).
- o_proj(qhalf 0) is interleaved into attention qhalf 1 (1 block per
  2 kt) to fill PE slack; the tail runs per-qq norm -> o_proj chains.
- Normalization: denom row (ones-column of the M=65 AV) -> DRAM
  round-trip broadcast -> reciprocal_approx_fast; gate fused as
  (tanh+1)*recip in one scalar_tensor_tensor (sigmoid(u) =
  0.5*(tanh(u/2)+1); the 0.5 is folded into w_o on host).
"""
import sys
import numpy as np

try:
    import concourse.bass as bass
except ImportError:
    sys.path.insert(0, "/opt/trn_rl_repo")
    import concourse.bass as bass

import concourse.tile as tile
from concourse import bacc, mybir
from concourse.bass_utils import run_bass_kernel_spmd

B, L, E, H = 1, 2048, 1024, 16
HW = E // H                # 64
SCALE = HW ** -0.5
N_CORES = 8
HPC = H // N_CORES         # 2 heads per core
C2 = HPC * HW              # 128
MASK_NEG = -60.0

f32 = mybir.dt.float32
f16 = mybir.dt.float16

NE = E // 128              # 8 contraction chunks
NKT = L // 128             # 16 key chunks of 128
KT_PE = (2, 5)             # per-qhalf: key chunks whose bias goes via PE
AV_LAG = 2                 # kt lag between scores and AV matmuls
# log-domain shifts so p = exp(s + b - 10*ln2) never overflows f16
C_EXP = float(6 * np.log(2.0))   # applied inside the Exp activation
C_BM = float(4 * np.log(2.0))    # applied to the bias on host

_compiled = [None]


def _build():
    nc = bacc.Bacc("TRN2", target_bir_lowering=False, debug=False,
                   num_devices=N_CORES)

    xT_ap = nc.dram_tensor("xT", [E, L], f16, kind="ExternalInput").ap()
    wpT_ap = nc.dram_tensor("wpT", [E, 3 * C2], f16, kind="ExternalInput").ap()
    wgT_ap = nc.dram_tensor("wgT", [E, C2], f16, kind="ExternalInput").ap()
    bgt_ap = nc.dram_tensor("bgt", [C2, 1], f32, kind="ExternalInput").ap()
    woT_ap = nc.dram_tensor("woT", [C2, E], f16, kind="ExternalInput").ap()
    bm_ap = nc.dram_tensor("bm", [NKT, 128, HPC, L], f16,
                           kind="ExternalInput").ap()
    ident_ap = nc.dram_tensor("ident", [128, 128], f16, kind="ExternalInput").ap()
    ones_ap = nc.dram_tensor("onescols", [128, NKT * 2], f16,
                             kind="ExternalInput").ap()
    outT_ap = nc.dram_tensor("outT", [E, L], f16, kind="ExternalOutput").ap()

    AOP = mybir.AluOpType
    EXP = mybir.ActivationFunctionType.Exp

    with tile.TileContext(nc) as tc:
        from contextlib import ExitStack
        with ExitStack() as ctx:
            pers = ctx.enter_context(tc.tile_pool(name="pers", bufs=1))
            biasp = ctx.enter_context(tc.tile_pool(name="bias", bufs=6))
            pp = ctx.enter_context(tc.tile_pool(name="pp", bufs=10))
            pep = ctx.enter_context(tc.tile_pool(name="pep", bufs=4))
            nrm = ctx.enter_context(tc.tile_pool(name="nrm", bufs=2))
            dramp = ctx.enter_context(tc.tile_pool(name="dram", bufs=4, space="DRAM"))
            outp = ctx.enter_context(tc.tile_pool(name="outp", bufs=4))
            # PSUM: 8 banks = s(2 bufs x 2 banks) + y0/y1(1 buf x 2 banks each)
            sp = ctx.enter_context(tc.tile_pool(name="s", bufs=2, space="PSUM"))
            yp = ctx.enter_context(tc.tile_pool(name="y", bufs=1, space="PSUM"))

            # ---- input DMAs (sync queue): ones first (tiny RMW writes must
            # land before the v transpose copies), then proj-critical tensors
            v_all_early = pers.tile([128, NKT, 2, 80], f16, tag="v_all")
            nc.gpsimd.memset(v_all_early[:, :, :, 64:65], 1.0)
            wpT_sb = [pers.tile([128, 3 * C2], f16, name=f"wpT{e}", tag=f"wpT{e}")
                      for e in range(NE)]
            xT_sb = [pers.tile([128, L], f16, name=f"xT{e}", tag=f"xT{e}")
                     for e in range(NE)]
            for e in range(NE):
                nc.sync.dma_start(wpT_sb[e], wpT_ap[e * 128:(e + 1) * 128, :])
                nc.sync.dma_start(xT_sb[e][:, 0:1024],
                                  xT_ap[e * 128:(e + 1) * 128, 0:1024])
            ident_sb = pers.tile([128, 128], f16, tag="ident")
            nc.sync.dma_start(ident_sb, ident_ap)
            for e in range(NE):
                nc.sync.dma_start(xT_sb[e][:, 1024:2048],
                                  xT_ap[e * 128:(e + 1) * 128, 1024:2048])
            wgT_sb = [pers.tile([128, C2], f16, name=f"wgT{e}", tag=f"wgT{e}")
                      for e in range(NE)]
            for e in range(NE):
                nc.sync.dma_start(wgT_sb[e], wgT_ap[e * 128:(e + 1) * 128, :])
            bgt_sb = pers.tile([C2, 1], f32, tag="bgt")
            nc.sync.dma_start(bgt_sb, bgt_ap)
            woT_sb = pers.tile([C2, E], f16, tag="woT")
            nc.sync.dma_start(woT_sb, woT_ap)
            # v layout [128 keys, kt, 136]: h0 = [v 0:64 | ones 64], h1 =
            # [v 68:132 | ones 132]; pads keep the ones columns on their own
            # 8-byte lines (the sub-512B ones-DMA does read-modify-write and
            # races DVE copies that share a line). Ones DMAs issued first.
            v_all = v_all_early

            # Act spline-table warmup (exp_and_others: Exp + Tanh)
            warm = pers.tile([C2, 1], f32, tag="warm")
            nc.scalar.activation(warm, bgt_sb, EXP)
            cexp_sb = pers.tile([128, 1], f32, tag="cexp")
            nc.gpsimd.memset(cexp_sb, -C_EXP)

            q01 = pers.tile([128, L], f16, tag="q01")
            k01 = pers.tile([128, L], f16, tag="k01")
            vT01 = pers.tile([128, L], f16, tag="vT01")
            g01 = pers.tile([128, L], f16, tag="g01")
            ygT = pers.tile([128, L], f16, tag="ygT")

            # ---------------- proj ----------------
            dests = {0: q01, 1: k01, 2: vT01}

            def proj_lh(f, lh, drain_eng=None):
                ps = sp.tile([128, 2, 512], f32, name=f"pj{f}_{lh}", tag="s")
                for e in range(NE):
                    w = wpT_sb[e][:, f * 128:(f + 1) * 128]
                    for ltq in range(2):
                        nc.tensor.matmul(
                            ps[:, ltq, :], w,
                            xT_sb[e][:, lh * 1024 + ltq * 512:
                                     lh * 1024 + (ltq + 1) * 512],
                            start=(e == 0), stop=(e == NE - 1))
                nc.vector.tensor_copy(
                    dests[f][:, lh * 1024:(lh + 1) * 1024], ps)

            def gate_lh(lh):
                ps = sp.tile([128, 2, 512], f32, name=f"pg{lh}", tag="s")
                for e in range(NE):
                    for ltq in range(2):
                        nc.tensor.matmul(
                            ps[:, ltq, :], wgT_sb[e],
                            xT_sb[e][:, lh * 1024 + ltq * 512:
                                     lh * 1024 + (ltq + 1) * 512],
                            start=(e == 0), stop=(e == NE - 1))
                nc.scalar.activation(
                    g01[:, lh * 1024:(lh + 1) * 1024], ps,
                    mybir.ActivationFunctionType.Tanh,
                    bias=bgt_sb, scale=0.5)

            # v (both halves) -> transposes; k (both); q lh0 only (qh0's
            # scores need q cols 0:1024; q-lh1 + gate interleave into qh0)
            def vtrans(kt):
                kts = slice(kt * 128, (kt + 1) * 128)
                tr = sp.tile([128, 128], f16, name=f"tr{kt}", tag="s")
                nc.tensor.transpose(tr, vT01[:, kts], ident_sb)
                nc.vector.tensor_copy(v_all[:, kt, 0, 0:64], tr[:, 0:64])
                nc.vector.tensor_copy(v_all[:, kt, 1, 0:64], tr[:, 64:128])

            proj_lh(2, 0)
            proj_lh(2, 1)
            proj_lh(1, 0)
            proj_lh(1, 1)
            proj_lh(0, 0)


            # ---------------- attention ----------------
            def emit_avs(y_t, pl, k0):
                for qq in range(2):
                    for h in range(2):
                        nc.tensor.matmul(
                            y_t[qq][:, h, :],
                            v_all[:, k0, h, 0:65],
                            pl[qq][:, h, :],
                            start=(k0 == 0), stop=(k0 == NKT - 1))

            def attention_pass(qhalf, extras=None, pre=None):
                y_t = [yp.tile([65, 2, 512], f32, name=f"y{qhalf}_{qq}",
                               tag=f"y{qq}") for qq in range(2)]
                pend = []
                for kt in range(NKT):
                    if pre is not None and kt in pre:
                        pre[kt]()
                    kts = slice(kt * 128, (kt + 1) * 128)
                    bt = biasp.tile([128, 2, 1024], f16,
                                    name=f"bt{qhalf}_{kt}", tag="bias")
                    nc.gpsimd.dma_start(
                        bt, bm_ap[kt, :, :, qhalf * 1024:(qhalf + 1) * 1024])
                    p_kt = []
                    for qq in range(2):
                        qs = slice(qhalf * 1024 + qq * 512,
                                   qhalf * 1024 + (qq + 1) * 512)
                        s_t = sp.tile([128, 2, 512], f32,
                                      name=f"s{qhalf}_{kt}_{qq}", tag="s")
                        pe_bias = kt < KT_PE[qhalf]
                        if pe_bias:
                            for h in range(2):
                                nc.tensor.matmul(
                                    s_t[:, h, :], ident_sb,
                                    bt[:, h, qq * 512:(qq + 1) * 512],
                                    start=True, stop=False)
                        for h in range(2):
                            hb = h * 64
                            nc.tensor.matmul(
                                s_t[:, h, :],
                                k01[hb:hb + 64, kts],
                                q01[hb:hb + 64, qs],
                                start=not pe_bias, stop=True)
                        p_t = pp.tile([128, 2, 512], f16,
                                      name=f"p{qhalf}_{kt}_{qq}", tag="p")
                        if pe_bias:
                            nc.scalar.activation(p_t, s_t, EXP)
                        else:
                            pe_t = pep.tile([128, 2, 512], f16,
                                            name=f"pe{qhalf}_{kt}_{qq}", tag="pe")
                            nc.scalar.activation(pe_t, s_t, EXP, bias=cexp_sb)
                            nc.vector.tensor_mul(
                                p_t, pe_t, bt[:, :, qq * 512:(qq + 1) * 512])
                        p_kt.append(p_t)
                    if extras is not None and kt in extras:
                        extras[kt]()
                    pend.append((p_kt, kt))
                    if len(pend) > AV_LAG:
                        pl, k0 = pend.pop(0)
                        emit_avs(y_t, pl, k0)
                for pl, k0 in pend:
                    emit_avs(y_t, pl, k0)
                return y_t

            def norm_rt(qhalf, qq, y_t):
                sums = nrm.tile([65, 2, 512], f32, name=f"sm{qhalf}_{qq}",
                                tag="sums")
                nc.vector.tensor_copy(sums[64:65, :, :], y_t[64:65, :, :])
                dscr = dramp.tile([1, 2, 512], f32, name=f"dscr{qhalf}_{qq}",
                                  tag="dscr")
                nc.gpsimd.dma_start(dscr, sums[64:65, :, :])
                rbs = nrm.tile([128, 512], f32, name=f"rbs{qhalf}_{qq}", tag="rbs")
                nc.gpsimd.dma_start(
                    rbs[0:64, :], dscr[0:1, 0, :].partition_broadcast(64))
                nc.gpsimd.dma_start(
                    rbs[64:128, :], dscr[0:1, 1, :].partition_broadcast(64))
                return rbs

            def norm_rc(qhalf, qq, rbs):
                qt = qhalf * 2 + qq
                qsl = slice(qt * 512, (qt + 1) * 512)
                rb = nrm.tile([128, 512], f32, name=f"rb{qhalf}_{qq}", tag="rb")
                nc.vector.reciprocal_approx_fast(rb, rbs)
                # grb = (tanh + 1) * (1/denom); the 0.5 lives in woT
                grb = nrm.tile([128, 512], f16, name=f"grb{qhalf}_{qq}", tag="grb")
                nc.vector.scalar_tensor_tensor(
                    grb, g01[:, qsl], 1.0, rb, AOP.add, AOP.mult)
                return grb

            def norm_b(qhalf, qq, y_t, grb):
                qt = qhalf * 2 + qq
                qsl = slice(qt * 512, (qt + 1) * 512)
                nc.vector.tensor_mul(ygT[0:64, qsl], y_t[0:64, 0, :], grb[0:64, :])
                yg1 = nrm.tile([64, 512], f16, name=f"yg1{qhalf}_{qq}", tag="yg1")
                nc.vector.tensor_mul(yg1, y_t[0:64, 1, :], grb[64:128, :])
                nc.sync.dma_start(ygT[64:128, qsl], yg1)

            def oproj_block(qhalf, eo):
                # both qq halves of one eo slice: 2 MMs + drain + out-DMA
                ps = sp.tile([128, 2, 512], f32, name=f"po{qhalf}_{eo}", tag="s")
                for qq in range(2):
                    qt = qhalf * 2 + qq
                    nc.tensor.matmul(
                        ps[:, qq, :],
                        woT_sb[:, eo * 128:(eo + 1) * 128],
                        ygT[:, qt * 512:(qt + 1) * 512],
                        start=True, stop=True)
                ot = outp.tile([128, 2, 512], f16, name=f"ot{qhalf}_{eo}",
                               tag="ot")
                nc.vector.tensor_copy(ot, ps)
                nc.sync.dma_start(
                    outT_ap[eo * 128:(eo + 1) * 128,
                            qhalf * 1024:(qhalf + 1) * 1024], ot)

            def oproj_qq(qhalf, qq, eo):
                qt = qhalf * 2 + qq
                ps = sp.tile([128, 512], f32, name=f"pq{qhalf}_{qq}_{eo}",
                             tag="s")
                nc.tensor.matmul(
                    ps, woT_sb[:, eo * 128:(eo + 1) * 128],
                    ygT[:, qt * 512:(qt + 1) * 512], start=True, stop=True)
                ot = outp.tile([128, 512], f16, name=f"oq{qhalf}_{qq}_{eo}",
                               tag="ot")
                if eo % 2 == 0:
                    nc.vector.tensor_copy(ot, ps)
                else:
                    nc.scalar.copy(ot, ps)
                nc.sync.dma_start(
                    outT_ap[eo * 128:(eo + 1) * 128,
                            qt * 512:(qt + 1) * 512], ot)

            for kt in range(8):
                vtrans(kt)
            def trpair(a):
                vtrans(a); vtrans(a + 1)
            extras0 = {2: lambda: proj_lh(0, 1),
                       4: lambda: trpair(8),
                       5: lambda: trpair(10),
                       6: lambda: trpair(12),
                       7: lambda: trpair(14),
                       9: lambda: gate_lh(0)}
            y_q0 = attention_pass(0, extras=extras0)
            r00 = norm_rt(0, 0, y_q0[0])
            r01 = norm_rt(0, 1, y_q0[1])
            g00 = norm_rc(0, 0, r00)
            g01_ = norm_rc(0, 1, r01)
            norm_b(0, 0, y_q0[0], g00)
            norm_b(0, 1, y_q0[1], g01_)
            # o_proj(qh0) + gate lh1 interleaved into attention(qh1)
            extras = {5 + j: (lambda j=j: oproj_block(0, j))
                      for j in range(NE)}
            extras[2] = lambda: gate_lh(1)
            y_q1 = attention_pass(1, extras=extras)
            r10 = norm_rt(1, 0, y_q1[0])
            r11 = norm_rt(1, 1, y_q1[1])
            g10 = norm_rc(1, 0, r10)
            g11 = norm_rc(1, 1, r11)
            norm_b(1, 0, y_q1[0], g10)
            norm_b(1, 1, y_q1[1], g11)
            for eo in range(NE):
                oproj_qq(1, 0, eo)
                oproj_qq(1, 1, eo)

    nc.compile()
    return nc


def kernel(x, mask, bias, w_proj, w_o, b_o, w_g, b_g):
    x = np.asarray(x, dtype=np.float32)
    mask = np.asarray(mask)
    bias = np.asarray(bias, dtype=np.float32)
    w_proj = np.asarray(w_proj, dtype=np.float32)
    w_o = np.asarray(w_o, dtype=np.float32)
    b_o = np.asarray(b_o, dtype=np.float32)
    w_g = np.asarray(w_g, dtype=np.float32)
    b_g = np.asarray(b_g, dtype=np.float32)

    if _compiled[0] is None:
        _compiled[0] = _build()
    nc = _compiled[0]

    xT = np.ascontiguousarray(x[0].T).astype(np.float16)      # [E, L]
    mask_add = np.where(mask[0], 0.0, MASK_NEG).astype(np.float32)  # [L]
    ident = np.eye(128, dtype=np.float16)
    onescols = np.ones((128, NKT * 2), dtype=np.float16)

    in_maps = []
    for c in range(N_CORES):
        heads = [c * HPC + i for i in range(HPC)]
        wpT = np.empty((E, 3 * C2), dtype=np.float16)
        for i, h in enumerate(heads):
            r0 = h * 3 * HW
            wpT[:, 0 * C2 + i * HW: 0 * C2 + (i + 1) * HW] = \
                (w_proj[r0: r0 + HW].T * SCALE).astype(np.float16)   # q
            wpT[:, 1 * C2 + i * HW: 1 * C2 + (i + 1) * HW] = \
                w_proj[r0 + HW: r0 + 2 * HW].T.astype(np.float16)    # k
            wpT[:, 2 * C2 + i * HW: 2 * C2 + (i + 1) * HW] = \
                w_proj[r0 + 2 * HW: r0 + 3 * HW].T.astype(np.float16)  # v
        # biasmix [NKT, 128, HPC, L]: raw bias (kt < KT_PE) else exp(bias)
        biasT = np.ascontiguousarray(
            bias[0, :, :, heads].transpose(0, 2, 1))          # [HPC, Lk, Lq]
        biasT += mask_add[None, :, None]
        bm = biasT.reshape(HPC, NKT, 128, L).transpose(1, 2, 0, 3)  # kt,k,h,q
        bmix = np.empty((NKT, 128, HPC, L), dtype=np.float16)
        for qh in range(2):
            qsl = slice(qh * 1024, (qh + 1) * 1024)
            kp = KT_PE[qh]
            bmix[:kp, :, :, qsl] = bm[:kp, :, :, qsl] - (C_EXP + C_BM)
            bmix[kp:, :, :, qsl] = np.exp(bm[kp:, :, :, qsl] - C_BM)
        cols = slice(c * C2, (c + 1) * C2)
        wgT = np.ascontiguousarray(w_g[cols, :].T).astype(np.float16)  # [E, C2]
        bgt = np.ascontiguousarray(0.5 * b_g[cols, None]).astype(np.float32)
        woT = np.ascontiguousarray(0.5 * w_o[:, cols].T).astype(np.float16)
        in_maps.append({
            "xT": xT, "wpT": wpT, "bm": bmix, "wgT": wgT,
            "bgt": bgt, "woT": woT, "ident": ident, "onescols": onescols,
        })

    res = run_bass_kernel_spmd(nc, in_maps, list(range(N_CORES)))
    acc = res.results[0]["outT"].astype(np.float64)
    for c in range(1, N_CORES):
        acc += res.results[c]["outT"]
    out = acc.T.astype(np.float32) + b_o[None, :]
    return out[None]  # [B, L, E]
